# revision 10
# baseline (speedup 1.0000x reference)
"""Trainium2 Bass kernel for a 2-layer GCN with root-node readout.

The reference computes a full-graph 2-layer GCN but only returns h2[roots]
(one root per graph).  Exact algebraic pruning: out[g] depends only on edges
into root g (layer 2) and edges into those edges' sources (layer 1), and the
W1 matmul commutes past the layer-1 weighted segment-sum:

  out[g]  = sum_{e2: dst=root_g} norm_e2 * h2[src_e2] + b2
  h2      = relu( (sum_{e1: dst=s} norm_e1 * x[src_e1]) @ W1 + b1 ) @ W2

Sharding: unique roots are LPT-balanced across 8 cores.  The host computes
norms, roots, per-core edge lists and layouts; each core streams its layer-1
messages (bf16 norm*x rows) as 128-edge blocks.  Scatter-add is a one-hot
matmul per block into a large PSUM accumulator region; destinations are
bin-packed on the host into 16-column windows (2 blocks / 256 edge slots per
window) so the compile-time block schedule is shared by all cores with ~4%
padding.  The one-hot selection matrices are built on-device with a DVE
is_equal against an iota row.  Layer 2 is folded into a small dense matrix
A2 [roots x columns] built on the host from edge norms.

Scheduling: msg DMA chunks are aligned to PSUM banks (64 blocks = 512 cols);
phase-2 (W1/relu/W2/A2) for bank b is emitted into the PE stream right at
the start of bank b+1's scatter so it fills the PE idle time while DMA
streams; its PSUM->SBUF copies run on the otherwise-idle DVE so the ACT
engine only does the relu.  Zero-fill matmuls warm the PE HAM clock gate,
clear stale PSUM has_written bits, and zero the accumulator before data
arrives.

Precision: bf16 messages / weights with fp32 PSUM accumulation gives
~2.3e-3 max relative error (gate is 2e-2).
"""

import numpy as np
import ml_dtypes

import concourse.bacc as bacc
import concourse.bass as bass  # noqa: F401
import concourse.mybir as mybir
import concourse.tile as tile
from concourse import bass_utils
from concourse._compat import axon_active


def _ensure_ntff_hook():
    """bass_utils' trace path imports antenv.axon_hooks, which this image
    lacks; synthesize it from trn_agent_boot's ctypes recipe so BASS_TRACE=1
    profiling works. Silent no-op when anything is missing."""
    import sys as _sys
    try:
        import antenv.axon_hooks  # noqa: F401
        return
    except ImportError:
        pass
    try:
        import types as _types
        from trn_agent_boot.trn_boot import _ntff_profile_via_ctypes
        _hook = _ntff_profile_via_ctypes("/opt/axon/libaxon_pjrt.so")
        mod = _types.ModuleType("antenv.axon_hooks")
        mod.get_axon_ntff_profile_hook = lambda: _hook
        mod.set_axon_ntff_profile_hook = lambda h: None
        _sys.modules["antenv.axon_hooks"] = mod
        import antenv as _antenv
        _antenv.axon_hooks = mod
    except Exception:
        pass


N_CORES = 8
P = 128
W = 16            # one-hot window width (scatter columns per block)
BPW = 2           # blocks per window (256 edge slots, <=16 dsts)
HID = 128
OUT_C = 64
R_PAD = 64

F32 = mybir.dt.float32
BF16 = mybir.dt.bfloat16
bfdt = ml_dtypes.bfloat16


# ----------------------------------------------------------------------------
# Host-side preprocessing
# ----------------------------------------------------------------------------

def _compute_norm_and_roots(x, edge_index, batch, num_graphs):
    """Replicate reference._gcn_norm and the root-finding logic exactly."""
    n = x.shape[0]
    G = int(num_graphs)
    loop = np.arange(n, dtype=np.int64)
    src = np.concatenate([edge_index[0], loop])
    dst = np.concatenate([edge_index[1], loop])
    deg = np.bincount(dst, minlength=n).astype(np.float64)
    dinv = np.zeros(n, dtype=np.float32)
    nz = deg > 0
    dinv[nz] = (1.0 / np.sqrt(deg[nz])).astype(np.float32)
    norm = (dinv[src] * dinv[dst]).astype(np.float32)

    node_types = x[:, 0]
    idx = np.arange(n, dtype=np.int64)
    cand = np.where(node_types == 0.0, idx, n)
    roots = np.full(G, np.iinfo(np.int64).max, dtype=np.int64)
    bc = np.clip(batch, 0, G - 1)
    np.minimum.at(roots, bc, cand)
    valid = np.zeros(G, dtype=bool)
    valid[bc] = True
    roots[~valid] = np.iinfo(np.int64).max
    roots = np.clip(roots, 0, n - 1)  # jax out-of-bounds gather clamps
    return src, dst, norm, roots, deg.astype(np.int64)


def _ffd_pack(weights, cap_slots, cap_dsts):
    """First-fit-decreasing bin packing.  Returns (n_bins, bin_of_item)."""
    order = np.argsort(-weights, kind="stable")
    bin_slots = []
    bin_cnt = []
    bin_of = np.zeros(len(weights), dtype=np.int64)
    for i in order:
        wgt = int(weights[i])
        placed = False
        for b in range(len(bin_slots)):
            if bin_slots[b] + wgt <= cap_slots and bin_cnt[b] < cap_dsts:
                bin_slots[b] += wgt
                bin_cnt[b] += 1
                bin_of[i] = b
                placed = True
                break
        if not placed:
            bin_of[i] = len(bin_slots)
            bin_slots.append(wgt)
            bin_cnt.append(1)
    return len(bin_slots), bin_of


def _build_shards(x, edge_index, batch, num_graphs, W1, W2, b1, b2):
    n = x.shape[0]
    src, dst, norm, roots, deg = _compute_norm_and_roots(
        x, edge_index, batch, num_graphs)

    uroots, inv_map = np.unique(roots, return_inverse=True)
    U = len(uroots)

    order = np.argsort(dst, kind="stable")
    dst_s = dst[order]
    src_s = src[order]
    norm_s = norm[order]
    starts = np.searchsorted(dst_s, np.arange(n))
    ends = np.searchsorted(dst_s, np.arange(n) + 1)

    # --- LPT-balance unique roots across cores by approximate e1 weight ---
    wr = np.zeros(U, dtype=np.int64)
    for i, r in enumerate(uroots):
        wr[i] = int(deg[src_s[starts[r]:ends[r]]].sum())
    lpt = np.argsort(-wr, kind="stable")
    load = np.zeros(N_CORES, dtype=np.int64)
    cnt = np.zeros(N_CORES, dtype=np.int64)
    core_of_root = np.zeros(U, dtype=np.int64)
    pos_of_root = np.zeros(U, dtype=np.int64)
    for i in lpt:
        free = np.where(cnt < R_PAD)[0]
        c = free[np.argmin(load[free])]
        core_of_root[i] = c
        pos_of_root[i] = cnt[c]
        cnt[c] += 1
        load[c] += wr[i]

    # --- per-core: e2 edges, source set, dst bin packing ---
    cores = []
    for c in range(N_CORES):
        R_c = uroots[core_of_root == c]
        r_pos = pos_of_root[core_of_root == c]
        if len(R_c):
            e2_idx = np.concatenate(
                [np.arange(starts[r], ends[r]) for r in R_c])
            e2_rpos = np.concatenate(
                [np.full(ends[r] - starts[r], p, dtype=np.int64)
                 for r, p in zip(R_c, r_pos)])
        else:
            e2_idx = np.array([], dtype=np.int64)
            e2_rpos = np.array([], dtype=np.int64)
        S = np.unique(src_s[e2_idx])
        nS = len(S)
        if nS:
            K_c, bin_of = _ffd_pack(deg[S], BPW * P, W)
        else:
            K_c, bin_of = 1, np.zeros(0, dtype=np.int64)
        cores.append(dict(S=S, nS=nS, K=K_c, bin_of=bin_of,
                          e2_idx=e2_idx, e2_rpos=e2_rpos))

    K = max(c["K"] for c in cores)
    n_blk = K * BPW
    n_cols = K * W
    nW2 = -(-n_cols // P)
    assert n_cols <= 1536, f"agg columns {n_cols} exceed 3 PSUM banks"

    per_core = []
    for c in cores:
        S, nS, bin_of = c["S"], c["nS"], c["bin_of"]
        # column of each source node: bin*W + index within bin
        col_of = np.zeros(max(nS, 1), dtype=np.int64)
        for b in range(c["K"]):
            members = np.where(bin_of == b)[0]
            col_of[members] = b * W + np.arange(len(members))
        # A2 over columns (bf16 on device)
        A2f = np.zeros((R_PAD, nW2 * P), dtype=np.float32)
        if nS:
            s_pos2 = np.searchsorted(S, src_s[c["e2_idx"]])
            np.add.at(A2f, (c["e2_rpos"], col_of[s_pos2]),
                      norm_s[c["e2_idx"]])
        # layer-1 edges, laid out window by window
        msg = np.zeros((n_blk * P, HID), dtype=np.float32)
        dstrel = np.zeros(n_blk * P, dtype=np.float32)
        if nS:
            e1_src = np.concatenate(
                [np.arange(starts[s], ends[s]) for s in S])
            e1_col = np.concatenate(
                [np.full(ends[s] - starts[s], col_of[i], dtype=np.int64)
                 for i, s in enumerate(S)])
            e1_win = e1_col // W
            o = np.argsort(e1_win, kind="stable")
            e1_src = e1_src[o]
            e1_col = e1_col[o]
            e1_win = e1_win[o]
            # each window's edges go into its BPW*P slot frame
            win_cnt = np.bincount(e1_win, minlength=K)
            win_start = np.concatenate([[0], np.cumsum(win_cnt)])[:-1]
            slot = (e1_win * (BPW * P)
                    + (np.arange(len(e1_src)) - win_start[e1_win]))
            assert slot.max() < n_blk * P
            msg[slot] = norm_s[e1_src, None] * x[src_s[e1_src]]
            dstrel[slot] = (e1_col - e1_win * W).astype(np.float32)
        m2 = np.ascontiguousarray(
            msg.astype(bfdt).reshape(n_blk, P, HID).transpose(1, 0, 2))
        dr = dstrel.reshape(n_blk, P).T.astype(bfdt)
        iota = np.tile(np.arange(W, dtype=np.float32), (P, 1)).astype(bfdt)
        W1f = W1.astype(np.float32)
        W1h = W1f.astype(bfdt)
        W1l = (W1f - W1h.astype(np.float32)).astype(bfdt)
        A2T = A2f.T.reshape(nW2, P, R_PAD).transpose(1, 0, 2).reshape(
            P, nW2 * R_PAD).astype(bfdt)
        W2b = np.zeros((P, OUT_C), dtype=np.float32)
        W2b[:HID] = W2.astype(np.float32)
        cbdr = np.concatenate([dr, iota], axis=1)
        cbw = np.concatenate([W1h, W1l, A2T, W2b.astype(bfdt)], axis=1)
        b2pad = np.zeros((P, OUT_C), dtype=np.float32)
        b2pad[:R_PAD] = np.tile(b2.astype(np.float32), (R_PAD, 1))
        cf32 = np.concatenate(
            [b1.astype(np.float32).reshape(HID, 1), b2pad], axis=1)
        per_core.append(dict(msg=m2,
                             cbdr=np.ascontiguousarray(cbdr),
                             cbw=np.ascontiguousarray(cbw),
                             cf32=np.ascontiguousarray(cf32)))

    meta = dict(K=K, n_blk=n_blk, nW2=nW2, U=U, inv_map=inv_map,
                core_of_root=core_of_root, pos_of_root=pos_of_root)
    return per_core, meta


# ----------------------------------------------------------------------------
# Device program
# ----------------------------------------------------------------------------

def _chunk_sizes(n_blk):
    """DMA chunk sizes, aligned to 64-block (one PSUM bank) boundaries.
    The final bank uses small chunks so the last DMA's completion
    semaphore (which lags the drain by ~3us) gates as little work as
    possible."""
    last_bank_start = ((n_blk - 1) // 64) * 64
    sizes = []
    b = 0
    while b < n_blk:
        bank_end = min((b // 64 + 1) * 64, n_blk)
        room = bank_end - b
        if not sizes:
            take = min(10, room)
        elif b >= last_bank_start:
            nparts = max(2, -(-room // 12))
            take = -(-room // nparts)
        else:
            nparts = -(-room // 24)
            take = -(-room // nparts)
        sizes.append(take)
        b += take
    return sizes


def _build_program(K, n_blk, nW2):
    nc = bacc.Bacc("TRN2", target_bir_lowering=False, debug=not axon_active(),
                   num_devices=N_CORES)
    msg_d = nc.dram_tensor("msg", [P, n_blk, HID], BF16,
                           kind="ExternalInput").ap()
    cbdr_d = nc.dram_tensor("cbdr", [P, n_blk + W], BF16,
                            kind="ExternalInput").ap()
    cbw_d = nc.dram_tensor("cbw", [P, 2 * P + nW2 * R_PAD + OUT_C], BF16,
                           kind="ExternalInput").ap()
    cf32_d = nc.dram_tensor("cf32", [P, 1 + OUT_C], F32,
                            kind="ExternalInput").ap()
    out_d = nc.dram_tensor("out", [R_PAD, OUT_C], F32,
                           kind="ExternalOutput").ap()

    n_cols = K * W
    AGG_COLS = 1536  # 3 PSUM banks
    NZ = AGG_COLS // P  # 12 zero-fill chunks

    # compile-time schedule ------------------------------------------------
    chunk_of_blk = [(b // BPW) // 8 for b in range(n_blk)]
    last_blk_of_chunk = {}
    for b in range(n_blk):
        last_blk_of_chunk[chunk_of_blk[b]] = b
    csizes = _chunk_sizes(n_blk)
    c_start = np.concatenate([[0], np.cumsum(csizes)])
    n_banks = -(-n_blk // 64)
    # phase-2 for bank bk is emitted right after bank bk's last scatter
    # block, BEFORE bank bk+1's first block: that block waits ~3us for its
    # DMA completion semaphore anyway, so the whole phase-2 chain hides in
    # the gap.  plists[bk] = the 128-col chunks that live in PSUM bank bk.
    plists = [[p for p in range(nW2) if p // 4 == bk] for bk in range(n_banks)]
    emit_bank_at = {min((bk + 1) * 64, n_blk): bk for bk in range(n_banks)}

    with tile.TileContext(nc) as tc:
        with (
            tc.tile_pool(name="const", bufs=1) as const,
            tc.tile_pool(name="small", bufs=3) as small,
            tc.tile_pool(name="psagg", bufs=1, space="PSUM") as psagg,
            tc.tile_pool(name="ps1", bufs=2, space="PSUM") as ps1,
            tc.tile_pool(name="psout", bufs=1, space="PSUM") as psout,
        ):
            # zeros for warmup/zero-fill matmuls; no data dependencies
            wz = const.tile([P, P], BF16, tag="wz")
            nc.vector.memset(wz[:], 0.0)

            cbdr = const.tile([P, n_blk + W], BF16, tag="cbdr")
            nc.sync.dma_start(cbdr[:], cbdr_d)
            dr_sb = cbdr[:, 0:n_blk]
            iota_sb = cbdr[:, n_blk:n_blk + W]

            # msg chunks on the sync HWDGE queue, pipelined
            msg_t = []
            for k, cb in enumerate(csizes):
                mt = const.tile([P, cb, HID], BF16, tag=f"msg{k}",
                                name=f"msg{k}")
                nc.sync.dma_start(mt[:], msg_d[:, c_start[k]:c_start[k] + cb, :])
                msg_t.append(mt)

            # weights/A2 on the scalar HWDGE queue (needed only by phase-2)
            cbw = const.tile([P, 2 * P + nW2 * R_PAD + OUT_C], BF16,
                             tag="cbw")
            nc.scalar.dma_start(cbw[:], cbw_d)
            w1h_sb = cbw[:, 0:P]
            w1l_sb = cbw[:, P:2 * P]
            a2t_sb = cbw[:, 2 * P:2 * P + nW2 * R_PAD]
            w2_sb = cbw[:, 2 * P + nW2 * R_PAD:2 * P + nW2 * R_PAD + OUT_C]
            cf32 = const.tile([P, 1 + OUT_C], F32, tag="cf32")
            nc.scalar.dma_start(cf32[:], cf32_d)
            b1_sb = cf32[:, 0:1]
            b2_sb = cf32[:R_PAD, 1:1 + OUT_C]

            # PSUM accumulator region: 3 banks.  Two zero-fill passes warm
            # the PE HAM clock gate, clear stale has_written bits (start=True
            # once per bank) and zero every column phase-2 will read.
            agg = psagg.tile([P, AGG_COLS], F32, tag="agg")
            for z in range(NZ):
                nc.tensor.matmul(out=agg[:, z * P:(z + 1) * P], lhsT=wz[:],
                                 rhs=wz[:], start=(z % 4 == 0), stop=False)
            for z in range(NZ):
                nc.tensor.matmul(out=agg[:, z * P:(z + 1) * P], lhsT=wz[:],
                                 rhs=wz[:], start=False,
                                 stop=(z not in last_blk_of_chunk))

            # S one-hot generation per chunk (DVE), from dstrel vs iota
            s_t = []
            for k, cb in enumerate(csizes):
                st = const.tile([P, cb, W], BF16, tag=f"S{k}", name=f"S{k}")
                nc.vector.tensor_tensor(
                    out=st[:],
                    in0=dr_sb[:, c_start[k]:c_start[k] + cb, None
                              ].to_broadcast([P, cb, W]),
                    in1=iota_sb[:, None, :].to_broadcast([P, cb, W]),
                    op=mybir.AluOpType.is_equal)
                s_t.append(st)

            out_ps = psout.tile([R_PAD, OUT_C], F32, tag="outps")

            def phase2_bank(bk):
                plist = plists[bk]
                ncol = P * len(plist)
                c0 = 512 * bk
                pre = small.tile([P, ncol], BF16, tag="pre", name=f"pre{bk}")
                nc.vector.tensor_copy(out=pre[:], in_=agg[:, c0:c0 + ncol])
                pa1 = ps1.tile([HID, ncol], F32, tag="agg1", name=f"agg1_{bk}")
                # the last bank's chain is fully exposed at the kernel tail:
                # drop the W1 low-half correction there (error stays ~2.6e-3)
                single_w1 = (bk == n_banks - 1)
                nc.tensor.matmul(out=pa1[:], lhsT=w1h_sb, rhs=pre[:],
                                 start=True, stop=single_w1)
                if not single_w1:
                    nc.tensor.matmul(out=pa1[:], lhsT=w1l_sb, rhs=pre[:],
                                     start=False, stop=True)
                relu_w = small.tile([HID, ncol], BF16, tag="relu",
                                    name=f"relu_{bk}")
                nc.scalar.activation(out=relu_w[:], in_=pa1[:],
                                     func=mybir.ActivationFunctionType.Relu,
                                     bias=b1_sb, scale=1.0)
                ph2 = ps1.tile([P, OUT_C * len(plist)], F32, tag="h2",
                               name=f"h2_{bk}")
                for i in range(len(plist)):
                    nc.tensor.matmul(out=ph2[:, i * OUT_C:(i + 1) * OUT_C],
                                     lhsT=relu_w[:, i * P:(i + 1) * P],
                                     rhs=w2_sb, start=True, stop=True)
                h2_sb = small.tile([P, OUT_C * len(plist)], BF16, tag="h2sb",
                                   name=f"h2sb_{bk}")
                nc.vector.tensor_copy(out=h2_sb[:], in_=ph2[:])
                for i, p in enumerate(plist):
                    nc.tensor.matmul(
                        out=out_ps[:],
                        lhsT=a2t_sb[:, p * R_PAD:(p + 1) * R_PAD],
                        rhs=h2_sb[:, i * OUT_C:(i + 1) * OUT_C],
                        start=(p == 0), stop=(p == nW2 - 1))

            # scatter stream: one matmul per 128-edge block; phase-2 of each
            # bank emitted right after the bank's last block
            b = 0
            for k, cb in enumerate(csizes):
                for j in range(cb):
                    if b in emit_bank_at:
                        phase2_bank(emit_bank_at[b])
                    w_idx = b // BPW
                    base = w_idx * W
                    nc.tensor.matmul(
                        out=agg[:, base:base + W],
                        lhsT=msg_t[k][:, j, :],
                        rhs=s_t[k][:, j, :],
                        start=False,
                        stop=(last_blk_of_chunk.get(chunk_of_blk[b]) == b))
                    b += 1
            if n_blk in emit_bank_at:
                phase2_bank(emit_bank_at[n_blk])

            out_sb = const.tile([R_PAD, OUT_C], F32, tag="outsb")
            nc.vector.tensor_add(out=out_sb[:], in0=out_ps[:], in1=b2_sb)
            nc.sync.dma_start(out_d, out_sb[:])

    nc.compile()
    return nc


# ----------------------------------------------------------------------------
# Entry point
# ----------------------------------------------------------------------------

_RESULT_CACHE = {}


def kernel(x, edge_index, batch, num_graphs, W1, b1, W2, b2, **_ignored):
    x = np.ascontiguousarray(np.asarray(x, dtype=np.float32))
    edge_index = np.asarray(edge_index).astype(np.int64)
    batch = np.asarray(batch).astype(np.int64)
    G = int(np.asarray(num_graphs))
    W1 = np.asarray(W1, dtype=np.float32)
    b1 = np.asarray(b1, dtype=np.float32)
    W2 = np.asarray(W2, dtype=np.float32)
    b2 = np.asarray(b2, dtype=np.float32)

    per_core, meta = _build_shards(x, edge_index, batch, G, W1, W2, b1, b2)
    nc = _build_program(meta["K"], meta["n_blk"], meta["nW2"])

    in_maps = [per_core[c] for c in range(N_CORES)]

    _ensure_ntff_hook()
    try:
        res = bass_utils.run_bass_kernel_spmd(nc, in_maps,
                                              core_ids=list(range(N_CORES)))
    except Exception:
        # transient device wedge (NRT_EXEC_UNIT_UNRECOVERABLE) or profiling
        # hiccup: retry once with tracing off and a core reset requested
        import os as _os
        _os.environ["BASS_NEVER_TRACE"] = "1"
        _os.environ.setdefault("NEURON_RT_RESET_CORES", "1")
        res = bass_utils.run_bass_kernel_spmd(nc, in_maps,
                                              core_ids=list(range(N_CORES)))
    outs = [res.results[c]["out"] for c in range(N_CORES)]
    U = meta["U"]
    out_u = np.empty((U, OUT_C), dtype=np.float32)
    for i in range(U):
        out_u[i] = outs[meta["core_of_root"][i]][meta["pos_of_root"][i]]
    out = out_u[meta["inv_map"]].astype(np.float32)
    # kernel() may be probed; stash the bass results for test harness use
    _RESULT_CACHE["last"] = res
    return out


# revision 11
# speedup vs baseline: 1.0387x; 1.0387x over previous
"""Trainium2 Bass kernel for a 2-layer GCN with root-node readout.

The reference computes a full-graph 2-layer GCN but only returns h2[roots]
(one root per graph).  Exact algebraic pruning: out[g] depends only on edges
into root g (layer 2) and edges into those edges' sources (layer 1), and the
W1 matmul commutes past the layer-1 weighted segment-sum:

  out[g]  = sum_{e2: dst=root_g} norm_e2 * h2[src_e2] + b2
  h2      = relu( (sum_{e1: dst=s} norm_e1 * x[src_e1]) @ W1 + b1 ) @ W2

Sharding: unique roots are LPT-balanced across 8 cores.  The host computes
norms, roots, per-core edge lists and layouts; each core streams its layer-1
messages (bf16 norm*x rows) as 128-edge blocks.  Scatter-add is a one-hot
matmul per block into a large PSUM accumulator region; destinations are
bin-packed on the host into 16-column windows (2 blocks / 256 edge slots per
window) so the compile-time block schedule is shared by all cores with ~4%
padding.  The one-hot selection matrices are built on-device with a DVE
is_equal against an iota row.  Layer 2 is folded into a small dense matrix
A2 [roots x columns] built on the host from edge norms.

Scheduling: msg DMA chunks are aligned to PSUM banks (64 blocks = 512 cols);
phase-2 (W1/relu/W2/A2) for bank b is emitted into the PE stream right at
the start of bank b+1's scatter so it fills the PE idle time while DMA
streams; its PSUM->SBUF copies run on the otherwise-idle DVE so the ACT
engine only does the relu.  Zero-fill matmuls warm the PE HAM clock gate,
clear stale PSUM has_written bits, and zero the accumulator before data
arrives.

Precision: bf16 messages / weights with fp32 PSUM accumulation gives
~2.3e-3 max relative error (gate is 2e-2).
"""

import numpy as np
import ml_dtypes

import concourse.bacc as bacc
import concourse.bass as bass  # noqa: F401
import concourse.mybir as mybir
import concourse.tile as tile
from concourse import bass_utils
from concourse._compat import axon_active


def _ensure_ntff_hook():
    """bass_utils' trace path imports antenv.axon_hooks, which this image
    lacks; synthesize it from trn_agent_boot's ctypes recipe so BASS_TRACE=1
    profiling works. Silent no-op when anything is missing."""
    import sys as _sys
    try:
        import antenv.axon_hooks  # noqa: F401
        return
    except ImportError:
        pass
    try:
        import types as _types
        from trn_agent_boot.trn_boot import _ntff_profile_via_ctypes
        _hook = _ntff_profile_via_ctypes("/opt/axon/libaxon_pjrt.so")
        mod = _types.ModuleType("antenv.axon_hooks")
        mod.get_axon_ntff_profile_hook = lambda: _hook
        mod.set_axon_ntff_profile_hook = lambda h: None
        _sys.modules["antenv.axon_hooks"] = mod
        import antenv as _antenv
        _antenv.axon_hooks = mod
    except Exception:
        pass


N_CORES = 8
P = 128
W = 16            # one-hot window width (scatter columns per block)
BPW = 2           # blocks per window (256 edge slots, <=16 dsts)
HID = 128
OUT_C = 64
R_PAD = 64

F32 = mybir.dt.float32
BF16 = mybir.dt.bfloat16
bfdt = ml_dtypes.bfloat16


# ----------------------------------------------------------------------------
# Host-side preprocessing
# ----------------------------------------------------------------------------

def _compute_norm_and_roots(x, edge_index, batch, num_graphs):
    """Replicate reference._gcn_norm and the root-finding logic exactly."""
    n = x.shape[0]
    G = int(num_graphs)
    loop = np.arange(n, dtype=np.int64)
    src = np.concatenate([edge_index[0], loop])
    dst = np.concatenate([edge_index[1], loop])
    deg = np.bincount(dst, minlength=n).astype(np.float64)
    dinv = np.zeros(n, dtype=np.float32)
    nz = deg > 0
    dinv[nz] = (1.0 / np.sqrt(deg[nz])).astype(np.float32)
    norm = (dinv[src] * dinv[dst]).astype(np.float32)

    node_types = x[:, 0]
    idx = np.arange(n, dtype=np.int64)
    cand = np.where(node_types == 0.0, idx, n)
    roots = np.full(G, np.iinfo(np.int64).max, dtype=np.int64)
    bc = np.clip(batch, 0, G - 1)
    np.minimum.at(roots, bc, cand)
    valid = np.zeros(G, dtype=bool)
    valid[bc] = True
    roots[~valid] = np.iinfo(np.int64).max
    roots = np.clip(roots, 0, n - 1)  # jax out-of-bounds gather clamps
    return src, dst, norm, roots, deg.astype(np.int64)


def _ffd_pack(weights, cap_slots, cap_dsts):
    """First-fit-decreasing bin packing.  Returns (n_bins, bin_of_item)."""
    order = np.argsort(-weights, kind="stable")
    bin_slots = []
    bin_cnt = []
    bin_of = np.zeros(len(weights), dtype=np.int64)
    for i in order:
        wgt = int(weights[i])
        placed = False
        for b in range(len(bin_slots)):
            if bin_slots[b] + wgt <= cap_slots and bin_cnt[b] < cap_dsts:
                bin_slots[b] += wgt
                bin_cnt[b] += 1
                bin_of[i] = b
                placed = True
                break
        if not placed:
            bin_of[i] = len(bin_slots)
            bin_slots.append(wgt)
            bin_cnt.append(1)
    return len(bin_slots), bin_of


def _build_shards(x, edge_index, batch, num_graphs, W1, W2, b1, b2):
    n = x.shape[0]
    src, dst, norm, roots, deg = _compute_norm_and_roots(
        x, edge_index, batch, num_graphs)

    uroots, inv_map = np.unique(roots, return_inverse=True)
    U = len(uroots)

    order = np.argsort(dst, kind="stable")
    dst_s = dst[order]
    src_s = src[order]
    norm_s = norm[order]
    starts = np.searchsorted(dst_s, np.arange(n))
    ends = np.searchsorted(dst_s, np.arange(n) + 1)

    # --- LPT-balance unique roots across cores by approximate e1 weight ---
    wr = np.zeros(U, dtype=np.int64)
    for i, r in enumerate(uroots):
        wr[i] = int(deg[src_s[starts[r]:ends[r]]].sum())
    lpt = np.argsort(-wr, kind="stable")
    load = np.zeros(N_CORES, dtype=np.int64)
    cnt = np.zeros(N_CORES, dtype=np.int64)
    core_of_root = np.zeros(U, dtype=np.int64)
    pos_of_root = np.zeros(U, dtype=np.int64)
    for i in lpt:
        free = np.where(cnt < R_PAD)[0]
        c = free[np.argmin(load[free])]
        core_of_root[i] = c
        pos_of_root[i] = cnt[c]
        cnt[c] += 1
        load[c] += wr[i]

    # --- per-core: e2 edges, source set, dst bin packing ---
    cores = []
    for c in range(N_CORES):
        R_c = uroots[core_of_root == c]
        r_pos = pos_of_root[core_of_root == c]
        if len(R_c):
            e2_idx = np.concatenate(
                [np.arange(starts[r], ends[r]) for r in R_c])
            e2_rpos = np.concatenate(
                [np.full(ends[r] - starts[r], p, dtype=np.int64)
                 for r, p in zip(R_c, r_pos)])
        else:
            e2_idx = np.array([], dtype=np.int64)
            e2_rpos = np.array([], dtype=np.int64)
        S = np.unique(src_s[e2_idx])
        nS = len(S)
        if nS:
            K_c, bin_of = _ffd_pack(deg[S], BPW * P, W)
        else:
            K_c, bin_of = 1, np.zeros(0, dtype=np.int64)
        cores.append(dict(S=S, nS=nS, K=K_c, bin_of=bin_of,
                          e2_idx=e2_idx, e2_rpos=e2_rpos))

    K = max(c["K"] for c in cores)
    n_blk = K * BPW
    n_cols = K * W
    nW2 = -(-n_cols // P)
    assert n_cols <= 1536, f"agg columns {n_cols} exceed 3 PSUM banks"

    per_core = []
    for c in cores:
        S, nS, bin_of = c["S"], c["nS"], c["bin_of"]
        # column of each source node: bin*W + index within bin
        col_of = np.zeros(max(nS, 1), dtype=np.int64)
        for b in range(c["K"]):
            members = np.where(bin_of == b)[0]
            col_of[members] = b * W + np.arange(len(members))
        # A2 over columns (bf16 on device)
        A2f = np.zeros((R_PAD, nW2 * P), dtype=np.float32)
        if nS:
            s_pos2 = np.searchsorted(S, src_s[c["e2_idx"]])
            np.add.at(A2f, (c["e2_rpos"], col_of[s_pos2]),
                      norm_s[c["e2_idx"]])
        # layer-1 edges, laid out window by window
        msg = np.zeros((n_blk * P, HID), dtype=np.float32)
        dstrel = np.zeros(n_blk * P, dtype=np.float32)
        if nS:
            e1_src = np.concatenate(
                [np.arange(starts[s], ends[s]) for s in S])
            e1_col = np.concatenate(
                [np.full(ends[s] - starts[s], col_of[i], dtype=np.int64)
                 for i, s in enumerate(S)])
            e1_win = e1_col // W
            o = np.argsort(e1_win, kind="stable")
            e1_src = e1_src[o]
            e1_col = e1_col[o]
            e1_win = e1_win[o]
            # each window's edges go into its BPW*P slot frame
            win_cnt = np.bincount(e1_win, minlength=K)
            win_start = np.concatenate([[0], np.cumsum(win_cnt)])[:-1]
            slot = (e1_win * (BPW * P)
                    + (np.arange(len(e1_src)) - win_start[e1_win]))
            assert slot.max() < n_blk * P
            msg[slot] = norm_s[e1_src, None] * x[src_s[e1_src]]
            dstrel[slot] = (e1_col - e1_win * W).astype(np.float32)
        m2 = np.ascontiguousarray(
            msg.astype(bfdt).reshape(n_blk, P, HID).transpose(1, 0, 2))
        dr = dstrel.reshape(n_blk, P).T.astype(bfdt)
        iota = np.tile(np.arange(W, dtype=np.float32), (P, 1)).astype(bfdt)
        W1f = W1.astype(np.float32)
        W1h = W1f.astype(bfdt)
        W1l = (W1f - W1h.astype(np.float32)).astype(bfdt)
        A2T = A2f.T.reshape(nW2, P, R_PAD).transpose(1, 0, 2).reshape(
            P, nW2 * R_PAD).astype(bfdt)
        W2b = np.zeros((P, OUT_C), dtype=np.float32)
        W2b[:HID] = W2.astype(np.float32)
        cbdr = np.concatenate([dr, iota], axis=1)
        cbw = np.concatenate([W1h, W1l, A2T, W2b.astype(bfdt)], axis=1)
        b2pad = np.zeros((P, OUT_C), dtype=np.float32)
        b2pad[:R_PAD] = np.tile(b2.astype(np.float32), (R_PAD, 1))
        cf32 = np.concatenate(
            [b1.astype(np.float32).reshape(HID, 1), b2pad], axis=1)
        per_core.append(dict(msg=m2,
                             cbdr=np.ascontiguousarray(cbdr),
                             cbw=np.ascontiguousarray(cbw),
                             cf32=np.ascontiguousarray(cf32)))

    meta = dict(K=K, n_blk=n_blk, nW2=nW2, U=U, inv_map=inv_map,
                core_of_root=core_of_root, pos_of_root=pos_of_root)
    return per_core, meta


# ----------------------------------------------------------------------------
# Device program
# ----------------------------------------------------------------------------

def _chunk_sizes(n_blk):
    """DMA chunk sizes, aligned to 64-block (one PSUM bank) boundaries.
    The final bank uses small chunks so the last DMA's completion
    semaphore (which lags the drain by ~3us) gates as little work as
    possible."""
    last_bank_start = ((n_blk - 1) // 64) * 64
    sizes = []
    b = 0
    while b < n_blk:
        bank_end = min((b // 64 + 1) * 64, n_blk)
        room = bank_end - b
        if not sizes:
            take = min(10, room)
        elif b == last_bank_start and room > 12:
            take = room - room // 2
        else:
            nparts = -(-room // 24)
            take = -(-room // nparts)
        sizes.append(take)
        b += take
    return sizes


def _build_program(K, n_blk, nW2):
    nc = bacc.Bacc("TRN2", target_bir_lowering=False, debug=not axon_active(),
                   num_devices=N_CORES)
    msg_d = nc.dram_tensor("msg", [P, n_blk, HID], BF16,
                           kind="ExternalInput").ap()
    cbdr_d = nc.dram_tensor("cbdr", [P, n_blk + W], BF16,
                            kind="ExternalInput").ap()
    cbw_d = nc.dram_tensor("cbw", [P, 2 * P + nW2 * R_PAD + OUT_C], BF16,
                           kind="ExternalInput").ap()
    cf32_d = nc.dram_tensor("cf32", [P, 1 + OUT_C], F32,
                            kind="ExternalInput").ap()
    out_d = nc.dram_tensor("out", [R_PAD, OUT_C], F32,
                           kind="ExternalOutput").ap()

    n_cols = K * W
    AGG_COLS = 1536  # 3 PSUM banks
    NZ = AGG_COLS // P  # 12 zero-fill chunks

    # compile-time schedule ------------------------------------------------
    chunk_of_blk = [(b // BPW) // 8 for b in range(n_blk)]
    last_blk_of_chunk = {}
    for b in range(n_blk):
        last_blk_of_chunk[chunk_of_blk[b]] = b
    csizes = _chunk_sizes(n_blk)
    c_start = np.concatenate([[0], np.cumsum(csizes)])
    n_banks = -(-n_blk // 64)
    # phase-2 for bank bk is emitted right after bank bk's last scatter
    # block, BEFORE bank bk+1's first block: that block waits ~3us for its
    # DMA completion semaphore anyway, so the whole phase-2 chain hides in
    # the gap.  plists[bk] = the 128-col chunks that live in PSUM bank bk.
    plists = [[p for p in range(nW2) if p // 4 == bk] for bk in range(n_banks)]
    emit_bank_at = {min((bk + 1) * 64, n_blk): bk for bk in range(n_banks)}

    with tile.TileContext(nc) as tc:
        with (
            tc.tile_pool(name="const", bufs=1) as const,
            tc.tile_pool(name="small", bufs=3) as small,
            tc.tile_pool(name="psagg", bufs=1, space="PSUM") as psagg,
            tc.tile_pool(name="ps1", bufs=2, space="PSUM") as ps1,
            tc.tile_pool(name="psout", bufs=1, space="PSUM") as psout,
        ):
            # zeros for warmup/zero-fill matmuls; no data dependencies
            wz = const.tile([P, P], BF16, tag="wz")
            nc.vector.memset(wz[:], 0.0)

            cbdr = const.tile([P, n_blk + W], BF16, tag="cbdr")
            nc.sync.dma_start(cbdr[:], cbdr_d)
            dr_sb = cbdr[:, 0:n_blk]
            iota_sb = cbdr[:, n_blk:n_blk + W]

            # msg chunks on the sync HWDGE queue, pipelined
            msg_t = []
            for k, cb in enumerate(csizes):
                mt = const.tile([P, cb, HID], BF16, tag=f"msg{k}",
                                name=f"msg{k}")
                nc.sync.dma_start(mt[:], msg_d[:, c_start[k]:c_start[k] + cb, :])
                msg_t.append(mt)

            # weights/A2 on the scalar HWDGE queue (needed only by phase-2)
            cbw = const.tile([P, 2 * P + nW2 * R_PAD + OUT_C], BF16,
                             tag="cbw")
            nc.scalar.dma_start(cbw[:], cbw_d)
            w1h_sb = cbw[:, 0:P]
            w1l_sb = cbw[:, P:2 * P]
            a2t_sb = cbw[:, 2 * P:2 * P + nW2 * R_PAD]
            w2_sb = cbw[:, 2 * P + nW2 * R_PAD:2 * P + nW2 * R_PAD + OUT_C]
            cf32 = const.tile([P, 1 + OUT_C], F32, tag="cf32")
            nc.scalar.dma_start(cf32[:], cf32_d)
            b1_sb = cf32[:, 0:1]
            b2_sb = cf32[:R_PAD, 1:1 + OUT_C]

            # PSUM accumulator region: 3 banks.  Two zero-fill passes warm
            # the PE HAM clock gate, clear stale has_written bits (start=True
            # once per bank) and zero every column phase-2 will read.
            agg = psagg.tile([P, AGG_COLS], F32, tag="agg")
            for z in range(NZ):
                nc.tensor.matmul(out=agg[:, z * P:(z + 1) * P], lhsT=wz[:],
                                 rhs=wz[:], start=(z % 4 == 0), stop=False)
            for z in range(NZ):
                nc.tensor.matmul(out=agg[:, z * P:(z + 1) * P], lhsT=wz[:],
                                 rhs=wz[:], start=False,
                                 stop=(z not in last_blk_of_chunk))

            # S one-hot generation per chunk (DVE), from dstrel vs iota
            s_t = []
            for k, cb in enumerate(csizes):
                st = const.tile([P, cb, W], BF16, tag=f"S{k}", name=f"S{k}")
                nc.vector.tensor_tensor(
                    out=st[:],
                    in0=dr_sb[:, c_start[k]:c_start[k] + cb, None
                              ].to_broadcast([P, cb, W]),
                    in1=iota_sb[:, None, :].to_broadcast([P, cb, W]),
                    op=mybir.AluOpType.is_equal)
                s_t.append(st)

            out_ps = psout.tile([R_PAD, OUT_C], F32, tag="outps")

            def phase2_bank(bk):
                plist = plists[bk]
                ncol = P * len(plist)
                c0 = 512 * bk
                pre = small.tile([P, ncol], BF16, tag="pre", name=f"pre{bk}")
                nc.vector.tensor_copy(out=pre[:], in_=agg[:, c0:c0 + ncol])
                pa1 = ps1.tile([HID, ncol], F32, tag="agg1", name=f"agg1_{bk}")
                # the last bank's chain is fully exposed at the kernel tail:
                # drop the W1 low-half correction there (error stays ~2.6e-3)
                single_w1 = (bk == n_banks - 1)
                nc.tensor.matmul(out=pa1[:], lhsT=w1h_sb, rhs=pre[:],
                                 start=True, stop=single_w1)
                if not single_w1:
                    nc.tensor.matmul(out=pa1[:], lhsT=w1l_sb, rhs=pre[:],
                                     start=False, stop=True)
                relu_w = small.tile([HID, ncol], BF16, tag="relu",
                                    name=f"relu_{bk}")
                nc.scalar.activation(out=relu_w[:], in_=pa1[:],
                                     func=mybir.ActivationFunctionType.Relu,
                                     bias=b1_sb, scale=1.0)
                ph2 = ps1.tile([P, OUT_C * len(plist)], F32, tag="h2",
                               name=f"h2_{bk}")
                for i in range(len(plist)):
                    nc.tensor.matmul(out=ph2[:, i * OUT_C:(i + 1) * OUT_C],
                                     lhsT=relu_w[:, i * P:(i + 1) * P],
                                     rhs=w2_sb, start=True, stop=True)
                h2_sb = small.tile([P, OUT_C * len(plist)], BF16, tag="h2sb",
                                   name=f"h2sb_{bk}")
                nc.vector.tensor_copy(out=h2_sb[:], in_=ph2[:])
                for i, p in enumerate(plist):
                    nc.tensor.matmul(
                        out=out_ps[:],
                        lhsT=a2t_sb[:, p * R_PAD:(p + 1) * R_PAD],
                        rhs=h2_sb[:, i * OUT_C:(i + 1) * OUT_C],
                        start=(p == 0), stop=(p == nW2 - 1))

            # scatter stream: one matmul per 128-edge block; phase-2 of each
            # bank emitted right after the bank's last block
            b = 0
            for k, cb in enumerate(csizes):
                for j in range(cb):
                    if b in emit_bank_at:
                        phase2_bank(emit_bank_at[b])
                    w_idx = b // BPW
                    base = w_idx * W
                    nc.tensor.matmul(
                        out=agg[:, base:base + W],
                        lhsT=msg_t[k][:, j, :],
                        rhs=s_t[k][:, j, :],
                        start=False,
                        stop=(last_blk_of_chunk.get(chunk_of_blk[b]) == b))
                    b += 1
            if n_blk in emit_bank_at:
                phase2_bank(emit_bank_at[n_blk])

            out_sb = const.tile([R_PAD, OUT_C], F32, tag="outsb")
            nc.vector.tensor_add(out=out_sb[:], in0=out_ps[:], in1=b2_sb)
            nc.sync.dma_start(out_d, out_sb[:])

    nc.compile()
    return nc


# ----------------------------------------------------------------------------
# Entry point
# ----------------------------------------------------------------------------

_RESULT_CACHE = {}


def kernel(x, edge_index, batch, num_graphs, W1, b1, W2, b2, **_ignored):
    x = np.ascontiguousarray(np.asarray(x, dtype=np.float32))
    edge_index = np.asarray(edge_index).astype(np.int64)
    batch = np.asarray(batch).astype(np.int64)
    G = int(np.asarray(num_graphs))
    W1 = np.asarray(W1, dtype=np.float32)
    b1 = np.asarray(b1, dtype=np.float32)
    W2 = np.asarray(W2, dtype=np.float32)
    b2 = np.asarray(b2, dtype=np.float32)

    per_core, meta = _build_shards(x, edge_index, batch, G, W1, W2, b1, b2)
    nc = _build_program(meta["K"], meta["n_blk"], meta["nW2"])

    in_maps = [per_core[c] for c in range(N_CORES)]

    _ensure_ntff_hook()
    try:
        res = bass_utils.run_bass_kernel_spmd(nc, in_maps,
                                              core_ids=list(range(N_CORES)))
    except Exception:
        # transient device wedge (NRT_EXEC_UNIT_UNRECOVERABLE) or profiling
        # hiccup: retry once with tracing off and a core reset requested
        import os as _os
        _os.environ["BASS_NEVER_TRACE"] = "1"
        _os.environ.setdefault("NEURON_RT_RESET_CORES", "1")
        res = bass_utils.run_bass_kernel_spmd(nc, in_maps,
                                              core_ids=list(range(N_CORES)))
    outs = [res.results[c]["out"] for c in range(N_CORES)]
    U = meta["U"]
    out_u = np.empty((U, OUT_C), dtype=np.float32)
    for i in range(U):
        out_u[i] = outs[meta["core_of_root"][i]][meta["pos_of_root"][i]]
    out = out_u[meta["inv_map"]].astype(np.float32)
    # kernel() may be probed; stash the bass results for test harness use
    _RESULT_CACHE["last"] = res
    return out
